# revision 1
# baseline (speedup 1.0000x reference)
"""AdaLoRA linear layer on 8 TRN2 NeuronCores.

Computes y = x @ (W + s * (P*Lambda*mask) @ Q)^T for
x[8192,4096], W[4096,4096], P[4096,64], Q[64,4096], s=2.0.

Strategy: data-parallel over the 8192 token dim (1024 tokens/core).
Each core computes its y shard with a single fused pass:
  t   = x_s @ Q^T                  (rank-64, tiny)
  y   = t_scaled @ P^T + x_s @ W^T (one PSUM accumulation group per tile)
All matmuls run in bf16 (f32 accumulation in PSUM); inputs are cast
f32->bf16 during the DMA into SBUF (SWDGE cast-DMA).

Host side passes transposed layouts (x^T, W^T, Q^T, P^T) so every DMA is
a natural contiguous load with the contraction dim on SBUF partitions.
"""

import os
import sys
import time
import types

for _p in ("/opt/trn_rl_repo", "/opt/pypackages"):
    if os.path.isdir(_p) and _p not in sys.path:
        sys.path.append(_p)

# antenv.axon_hooks is not shipped in this image, but bass_utils imports it
# when trace=True under axon. If it's genuinely missing, register a shim
# backed by the real ctypes NTFF hook so tracing still works.
try:
    import antenv.axon_hooks  # noqa: F401
except Exception:
    _mod = types.ModuleType("antenv.axon_hooks")
    _mod._hook = None

    def _set_hook(h, _m=_mod):
        _m._hook = h

    def _get_hook(_m=_mod):
        return _m._hook

    _mod.set_axon_ntff_profile_hook = _set_hook
    _mod.get_axon_ntff_profile_hook = _get_hook
    try:
        from trn_agent_boot.trn_boot import _ntff_profile_via_ctypes

        _mod._hook = _ntff_profile_via_ctypes("/opt/axon/libaxon_pjrt.so")
    except Exception:
        pass
    sys.modules["antenv.axon_hooks"] = _mod

import numpy as np

import concourse.mybir as mybir
import concourse.tile as tile
from concourse import bacc
from concourse.bass_utils import run_bass_kernel_spmd
from concourse.tile_rust import add_dep_helper

N_CORES = 8
IN_F = 4096
OUT_F = 4096
RANK = 64
BT = 8192
M_PER = BT // N_CORES  # 1024 tokens per core
SCALING = 2.0

P_DIM = 128
KB = IN_F // P_DIM  # 32 k-blocks
MS = M_PER // P_DIM  # 8 m-subtiles per core
N_STRIPE = 512
NS = OUT_F // N_STRIPE  # 8 n-stripes

_graph_cache = None


def _build_graph():
    f32 = mybir.dt.float32
    bf16 = mybir.dt.bfloat16
    u8 = mybir.dt.uint8

    nc = bacc.Bacc(None, target_bir_lowering=False, debug=False)

    xt = nc.declare_dram_parameter("xt", [IN_F, M_PER], f32, isOutput=False)
    wt = nc.declare_dram_parameter("wt", [IN_F, OUT_F], f32, isOutput=False)
    qtp = nc.declare_dram_parameter("qtp", [P_DIM, KB * RANK], f32, isOutput=False)
    pt = nc.declare_dram_parameter("pt", [RANK, OUT_F], f32, isOutput=False)
    lam = nc.declare_dram_parameter("lam", [RANK, 1], f32, isOutput=False)
    mask = nc.declare_dram_parameter("mask", [RANK, 1], u8, isOutput=False)
    out = nc.declare_dram_parameter("out", [M_PER, OUT_F], f32, isOutput=True)

    xt_r = xt[:].rearrange("(kb p) m -> p kb m", p=P_DIM)
    wt_r = wt[:].rearrange("(kb p) n -> p kb n", p=P_DIM)

    XH = 512  # x chunk width (m)
    NXC = M_PER // XH  # 2 chunks

    with tile.TileContext(nc) as tc:
        with (
            tc.tile_pool(name="const", bufs=1) as constp,
            tc.tile_pool(name="xpool", bufs=1) as xpool,
            tc.tile_pool(name="wpool", bufs=2) as wpool,
            tc.tile_pool(name="ypool", bufs=3) as ypool,
            tc.tile_pool(name="psum_y", bufs=6, space="PSUM") as psum_y_pool,
            tc.tile_pool(name="psum_t", bufs=2, space="PSUM") as psum_t_pool,
        ):
            # ---- constants ----
            qt_sb = constp.tile([P_DIM, KB, RANK], bf16)
            dma_qt = nc.gpsimd.dma_start(out=qt_sb[:], in_=qtp[:])

            # P^T zero-padded to 128 partitions (rank rows 64..127 are zero)
            pt_sb = constp.tile([P_DIM, OUT_F], bf16)
            nc.vector.memset(pt_sb[:], 0.0)
            dma_pt = nc.gpsimd.dma_start(out=pt_sb[0:RANK, :], in_=pt[:])

            lam_sb = constp.tile([RANK, 1], f32)
            nc.gpsimd.dma_start(out=lam_sb[:], in_=lam[:])
            mask_u8 = constp.tile([RANK, 1], u8)
            nc.gpsimd.dma_start(out=mask_u8[:], in_=mask[:])
            mask_f = constp.tile([RANK, 1], f32)
            nc.vector.tensor_copy(out=mask_f[:], in_=mask_u8[:])
            sv = constp.tile([RANK, 1], f32)
            nc.vector.tensor_mul(out=sv[:], in0=lam_sb[:], in1=mask_f[:])
            nc.scalar.mul(sv[:], sv[:], float(SCALING))

            # t^T (scaled) = sv * (x @ Q^T)^T, zero-padded to 128 partitions
            tT_all = constp.tile([P_DIM, NXC, XH], bf16)
            nc.vector.memset(tT_all[:], 0.0)

            # ---- x^T resident in SBUF (bf16), 2 m-chunks x 2 k-slabs ----
            KH = KB // 2
            xts = []
            xdmas = []  # per chunk [slabA, slabB]
            for h in range(NXC):
                xt_h = xpool.tile(
                    [P_DIM, KB, XH], bf16, name=f"xt_h{h}", tag=f"xt_h{h}"
                )
                ds = []
                for s_ in range(2):
                    ksl = slice(s_ * KH, (s_ + 1) * KH)
                    ds.append(
                        nc.gpsimd.dma_start(
                            out=xt_h[:, ksl, :],
                            in_=xt_r[:, ksl, h * XH : (h + 1) * XH],
                        )
                    )
                xts.append(xt_h)
                xdmas.append(ds)

            # ---- weight stripes (512 wide), each loaded as two k-slabs ----
            stripes = []  # (tile, width, col_offset)
            wdmas = []  # per-stripe [slabA, slabB]
            for ns in range(NS):
                off = ns * N_STRIPE
                wt_sb = wpool.tile(
                    [P_DIM, KB, N_STRIPE], bf16, tag="wt_sb", name=f"wt_sb{ns}"
                )
                ds = []
                for s_ in range(2):
                    ksl = slice(s_ * KH, (s_ + 1) * KH)
                    ds.append(
                        nc.gpsimd.dma_start(
                            out=wt_sb[:, ksl, :],
                            in_=wt_r[:, ksl, off : off + N_STRIPE],
                        )
                    )
                stripes.append((wt_sb, N_STRIPE, off))
                wdmas.append(ds)

            # DMA ordering: sliding window of 4 in-flight ~4MB transfers.
            # Pure racing equalizes completion times (everything lands at
            # the end); a serial chain can't saturate HBM bandwidth. A
            # shallow window keeps bandwidth saturated while preserving
            # approximate arrival order.
            seq = [dma_qt, *xdmas[0], wdmas[0][0], dma_pt, wdmas[0][1]]
            seq += [*xdmas[1], *wdmas[1]]
            for ns in range(2, NS):
                seq += wdmas[ns]
            WIN = 4
            for i in range(WIN, len(seq)):
                add_dep_helper(
                    seq[i].ins, seq[i - WIN].ins, reason="dma window order"
                )

            def t_phase(h):
                # t^T[:, h] = sv * (x_h @ Q^T)^T directly via matmul with
                # Q^T blocks stationary: out[r, m] in PSUM.
                psum_tT = psum_t_pool.tile([RANK, XH], f32, tag="psum_tT")
                for kb in range(KB):
                    nc.tensor.matmul(
                        psum_tT[:],
                        lhsT=qt_sb[:, kb, :],
                        rhs=xts[h][:, kb, :],
                        start=(kb == 0),
                        stop=(kb == KB - 1),
                    )
                nc.scalar.mul(tT_all[0:RANK, h, :], psum_tT[:], sv[:])

            def main_group(ws, ms):
                wt_sb, width, off = stripes[ws]
                h, mo = divmod(ms, XH // P_DIM)
                msl = slice(mo * P_DIM, (mo + 1) * P_DIM)
                ypsum = psum_y_pool.tile(
                    [P_DIM, N_STRIPE], f32, tag="ypsum", name="ypsum"
                )[:, :width]
                for kb in range(KB):
                    nc.tensor.matmul(
                        ypsum[:],
                        lhsT=xts[h][:, kb, msl],
                        rhs=wt_sb[:, kb, :],
                        start=(kb == 0),
                        stop=False,
                    )
                nc.tensor.matmul(
                    ypsum[:],
                    lhsT=tT_all[:, h, msl],
                    rhs=pt_sb[:, off : off + width],
                    start=False,
                    stop=True,
                )
                y_sb = ypool.tile(
                    [P_DIM, N_STRIPE], f32, tag="y_sb", name="y_sb"
                )[:, :width]
                nc.vector.tensor_copy(out=y_sb[:], in_=ypsum[:])
                nc.sync.dma_start(
                    out=out[ms * P_DIM : (ms + 1) * P_DIM, off : off + width],
                    in_=y_sb[:],
                )

            # PE order: t-phases as their x chunks land, stripe-0 groups
            # interleaved, then the remaining stripes.
            MPC = XH // P_DIM  # m-subtiles per x chunk
            t_phase(0)
            for ms in range(0, MPC):
                main_group(0, ms)
            t_phase(1)
            for ms in range(MPC, MS):
                main_group(0, ms)
            for ws in range(1, len(stripes)):
                for ms in range(MS):
                    main_group(ws, ms)

    nc.compile()
    return nc


def _get_graph():
    global _graph_cache
    if _graph_cache is None:
        _graph_cache = _build_graph()
    return _graph_cache


def run_full(inputs, trace=False, trace_kwargs=None):
    """Run the SPMD kernel on 8 cores. Returns (y_full, BassKernelResults)."""
    x = np.asarray(inputs["x"], dtype=np.float32)
    weight = np.asarray(inputs["weight"], dtype=np.float32)
    P = np.asarray(inputs["P"], dtype=np.float32)
    Lambda = np.asarray(inputs["Lambda"], dtype=np.float32)
    Q = np.asarray(inputs["Q"], dtype=np.float32)
    rank_mask = np.asarray(inputs["rank_mask"])

    xt = np.ascontiguousarray(x.T)  # [IN_F, BT]
    wt = np.ascontiguousarray(weight.T)  # [IN_F, OUT_F]
    # Q^T pre-tiled partition-major: [p, kb, r] flattened to [128, KB*RANK]
    qtp = np.ascontiguousarray(
        Q.T.reshape(KB, P_DIM, RANK).transpose(1, 0, 2).reshape(P_DIM, KB * RANK)
    )
    pt = np.ascontiguousarray(P.T)  # [RANK, OUT_F]
    lam = np.ascontiguousarray(Lambda.reshape(RANK, 1))
    mask_u8 = np.ascontiguousarray(rank_mask.reshape(RANK, 1).astype(np.uint8))

    in_maps = []
    for c in range(N_CORES):
        in_maps.append(
            {
                "xt": np.ascontiguousarray(xt[:, c * M_PER : (c + 1) * M_PER]),
                "wt": wt,
                "qtp": qtp,
                "pt": pt,
                "lam": lam,
                "mask": mask_u8,
            }
        )

    nc = _get_graph()
    last_err = None
    for attempt in range(3):
        try:
            res = run_bass_kernel_spmd(
                nc,
                in_maps,
                core_ids=list(range(N_CORES)),
                trace=trace,
                **(trace_kwargs or {}),
            )
            break
        except Exception as e:  # transient NRT device faults recover on retry
            last_err = e
            time.sleep(10)
    else:
        raise last_err
    y = np.concatenate([res.results[c]["out"] for c in range(N_CORES)], axis=0)
    return y.astype(np.float32, copy=False), res


def _device_available():
    try:
        import jax

        return any("NC" in str(d) or "axon" in str(d).lower() for d in jax.devices())
    except Exception:
        return False


def _run_in_subprocess(inputs):
    # The caller's process may have initialized jax on another platform
    # (e.g. JAX_PLATFORMS=cpu for the reference); run the device pass in a
    # clean child process where jax can pick up the axon/neuron backend.
    import pickle
    import subprocess
    import tempfile

    with tempfile.TemporaryDirectory() as td:
        in_path = os.path.join(td, "in.pkl")
        out_path = os.path.join(td, "out.npy")
        with open(in_path, "wb") as f:
            pickle.dump({k: np.asarray(v) for k, v in inputs.items()}, f)
        env = dict(os.environ)
        env.pop("JAX_PLATFORMS", None)
        env["KERNEL_NO_SUBPROC"] = "1"
        code = (
            "import sys, pickle, numpy as np; "
            f"sys.path.insert(0, {os.path.dirname(os.path.abspath(__file__))!r}); "
            "import kernel; "
            f"inputs = pickle.load(open({in_path!r}, 'rb')); "
            "y, _ = kernel.run_full(inputs, trace=False); "
            f"np.save({out_path!r}, y)"
        )
        subprocess.run([sys.executable, "-c", code], env=env, check=True)
        return np.load(out_path)


def kernel(**inputs) -> np.ndarray:
    if os.environ.get("KERNEL_NO_SUBPROC") != "1":
        if not _device_available():
            return _run_in_subprocess(inputs)
        try:
            y, _ = run_full(inputs, trace=False)
            return y
        except Exception:
            # A wedged device / PJRT client recovers in a fresh process
            # (observed empirically); retry once out-of-process.
            return _run_in_subprocess(inputs)
    y, _ = run_full(inputs, trace=False)
    return y



# revision 3
# speedup vs baseline: 1.0897x; 1.0897x over previous
"""AdaLoRA linear layer on 8 TRN2 NeuronCores.

Computes y = x @ (W + s * (P*Lambda*mask) @ Q)^T for
x[8192,4096], W[4096,4096], P[4096,64], Q[64,4096], s=2.0.

Strategy: data-parallel over the 8192 token dim (1024 tokens/core).
Each core computes its y shard with a single fused pass:
  t   = x_s @ Q^T                  (rank-64, tiny)
  y   = t @ Ptilde^T + x_s @ W^T   (one PSUM accumulation group per tile)
where Ptilde = P * (s*Lambda*mask) is folded on the host.

All device inputs are pre-cast to bf16 AND pre-tiled on the host into
the exact SBUF layout (partition-major [p, kb, free]) so every DMA is a
fat contiguous copy at full HBM bandwidth. f32 would double the DMA
bytes and starve the PE during warmup (measured: 27us head + 30us of
startup gaps + HAM clock-throttle until ~95us).
"""

import os
import sys
import time
import types

for _p in ("/opt/trn_rl_repo", "/opt/pypackages"):
    if os.path.isdir(_p) and _p not in sys.path:
        sys.path.append(_p)

# antenv.axon_hooks is not shipped in this image, but bass_utils imports it
# when trace=True under axon. If it's genuinely missing, register a shim
# backed by the real ctypes NTFF hook so tracing still works.
try:
    import antenv.axon_hooks  # noqa: F401
except Exception:
    _mod = types.ModuleType("antenv.axon_hooks")
    _mod._hook = None

    def _set_hook(h, _m=_mod):
        _m._hook = h

    def _get_hook(_m=_mod):
        return _m._hook

    _mod.set_axon_ntff_profile_hook = _set_hook
    _mod.get_axon_ntff_profile_hook = _get_hook
    try:
        from trn_agent_boot.trn_boot import _ntff_profile_via_ctypes

        _mod._hook = _ntff_profile_via_ctypes("/opt/axon/libaxon_pjrt.so")
    except Exception:
        pass
    sys.modules["antenv.axon_hooks"] = _mod

import ml_dtypes
import numpy as np

import concourse.mybir as mybir
import concourse.tile as tile
from concourse import bacc
from concourse.bass_utils import run_bass_kernel_spmd
from concourse.tile_rust import add_dep_helper

N_CORES = 8
IN_F = 4096
OUT_F = 4096
RANK = 64
BT = 8192
M_PER = BT // N_CORES  # 1024 tokens per core
SCALING = 2.0

P_DIM = 128
KB = IN_F // P_DIM  # 32 k-blocks
MS = M_PER // P_DIM  # 8 m-subtiles per core
N_STRIPE = 512
NS = OUT_F // N_STRIPE  # 8 n-stripes

XH = 512  # x chunk width (m)
NXC = M_PER // XH  # 2 chunks
MPC = XH // P_DIM  # 4 m-subtiles per x chunk

BF16 = ml_dtypes.bfloat16

_graph_cache = None


def _build_graph():
    f32 = mybir.dt.float32
    bf16 = mybir.dt.bfloat16

    nc = bacc.Bacc(None, target_bir_lowering=False, debug=False)

    # All inputs pre-tiled host-side, bf16.
    xt = nc.declare_dram_parameter("xt", [NXC, P_DIM, KB, XH], bf16, isOutput=False)
    wt = nc.declare_dram_parameter("wt", [NS, P_DIM, KB, N_STRIPE], bf16, isOutput=False)
    qtp = nc.declare_dram_parameter("qtp", [P_DIM, KB, RANK], bf16, isOutput=False)
    pt = nc.declare_dram_parameter("pt", [P_DIM, OUT_F], bf16, isOutput=False)
    out = nc.declare_dram_parameter("out", [M_PER, OUT_F], f32, isOutput=True)

    with tile.TileContext(nc) as tc:
        with (
            tc.tile_pool(name="const", bufs=1) as constp,
            tc.tile_pool(name="xpool", bufs=1) as xpool,
            tc.tile_pool(name="wpool", bufs=2) as wpool,
            tc.tile_pool(name="ypool", bufs=3) as ypool,
            tc.tile_pool(name="psum_y", bufs=6, space="PSUM") as psum_y_pool,
            tc.tile_pool(name="psum_t", bufs=2, space="PSUM") as psum_t_pool,
        ):
            # ---- constants ----
            qt_sb = constp.tile([P_DIM, KB, RANK], bf16)
            dma_qt = nc.gpsimd.dma_start(out=qt_sb[:], in_=qtp[:])

            # Ptilde^T, host-zero-padded to 128 partitions (rows 64..127)
            pt_sb = constp.tile([P_DIM, OUT_F], bf16)
            dma_pt = nc.gpsimd.dma_start(out=pt_sb[:], in_=pt[:])

            # t^T per chunk, zero-padded to 128 partitions
            tT_all = constp.tile([P_DIM, NXC, XH], bf16)
            nc.vector.memset(tT_all[:], 0.0)

            # ---- x^T resident in SBUF, 2 m-chunks; chunk0 in 4 kb-slabs
            # (fine-grained so compute starts as soon as slabs land) ----
            xts = []
            xdmas = []  # per chunk, list of slab dmas
            for h in range(NXC):
                xt_h = xpool.tile(
                    [P_DIM, KB, XH], bf16, name=f"xt_h{h}", tag=f"xt_h{h}"
                )
                nslab = 4 if h == 0 else 2
                step = KB // nslab
                ds = []
                for s_ in range(nslab):
                    ksl = slice(s_ * step, (s_ + 1) * step)
                    ds.append(
                        nc.gpsimd.dma_start(
                            out=xt_h[:, ksl, :], in_=xt[h, :, ksl, :]
                        )
                    )
                xts.append(xt_h)
                xdmas.append(ds)

            # ---- weight stripes (512 wide), 2 kb-slabs each ----
            stripes = []
            wdmas = []
            for ns in range(NS):
                wt_sb = wpool.tile(
                    [P_DIM, KB, N_STRIPE], bf16, tag="wt_sb", name=f"wt_sb{ns}"
                )
                nslab = 4 if ns == 0 else 2
                step = KB // nslab
                ds = []
                for s_ in range(nslab):
                    ksl = slice(s_ * step, (s_ + 1) * step)
                    ds.append(
                        nc.gpsimd.dma_start(
                            out=wt_sb[:, ksl, :], in_=wt[ns, :, ksl, :]
                        )
                    )
                stripes.append(wt_sb)
                wdmas.append(ds)

            # DMA ordering: sliding window keeps HBM saturated while
            # delivering the startup-critical slabs first.
            seq = [
                xdmas[0][0], xdmas[0][1], wdmas[0][0], dma_qt,
                xdmas[0][2], xdmas[0][3], wdmas[0][1], wdmas[0][2],
                wdmas[0][3],
                xdmas[1][0], wdmas[1][0], xdmas[1][1], wdmas[1][1],
                dma_pt,
            ]
            for ns in range(2, NS):
                seq += wdmas[ns]
            WIN = 5
            for i in range(WIN, len(seq)):
                add_dep_helper(
                    seq[i].ins, seq[i - WIN].ins, reason="dma window order"
                )

            # ---- compute ----
            t_psums = [None, None]

            def t_half(h, half):
                # t^T[:, h] partial: contraction over kb half into psum.
                if half == 0:
                    t_psums[h] = psum_t_pool.tile(
                        [RANK, XH], f32, tag="psum_tT", name=f"psum_tT{h}"
                    )
                psum_tT = t_psums[h]
                for kb in range(half * (KB // 2), (half + 1) * (KB // 2)):
                    nc.tensor.matmul(
                        psum_tT[:],
                        lhsT=qt_sb[:, kb, :],
                        rhs=xts[h][:, kb, :],
                        start=(kb == 0),
                        stop=(kb == KB - 1),
                    )
                if half == 1:
                    nc.scalar.copy(out=tT_all[0:RANK, h, :], in_=psum_tT[:])

            def mg_k(ws, ms, ypsum, k0, k1):
                h = ms // MPC
                mo = ms % MPC
                msl = slice(mo * P_DIM, (mo + 1) * P_DIM)
                for kb in range(k0, k1):
                    nc.tensor.matmul(
                        ypsum[:],
                        lhsT=xts[h][:, kb, msl],
                        rhs=stripes[ws][:, kb, :],
                        start=(kb == 0),
                        stop=False,
                    )

            def mg_tail(ws, ms, ypsum):
                h = ms // MPC
                mo = ms % MPC
                msl = slice(mo * P_DIM, (mo + 1) * P_DIM)
                off = ws * N_STRIPE
                nc.tensor.matmul(
                    ypsum[:],
                    lhsT=tT_all[:, h, msl],
                    rhs=pt_sb[:, off : off + N_STRIPE],
                    start=False,
                    stop=True,
                )
                y_sb = ypool.tile([P_DIM, N_STRIPE], f32, tag="y_sb", name="y_sb")
                nc.vector.tensor_copy(out=y_sb[:], in_=ypsum[:])
                nc.sync.dma_start(
                    out=out[ms * P_DIM : (ms + 1) * P_DIM, off : off + N_STRIPE],
                    in_=y_sb[:],
                )

            def new_ypsum():
                return psum_y_pool.tile(
                    [P_DIM, N_STRIPE], f32, tag="ypsum", name="ypsum"
                )

            def mg_full(ws, ms):
                yp = new_ypsum()
                mg_k(ws, ms, yp, 0, KB)
                mg_tail(ws, ms, yp)

            # PE order: interleave group(0,0) k-halves with t-phase(0)
            # halves so the first matmul only needs x0 kb0-15 + w0 kb0-15;
            # t-phase(1) rides along with group(0,1).
            yp00 = new_ypsum()
            mg_k(0, 0, yp00, 0, KB // 2)
            t_half(0, 0)
            mg_k(0, 0, yp00, KB // 2, KB)
            t_half(0, 1)
            mg_tail(0, 0, yp00)

            yp01 = new_ypsum()
            mg_k(0, 1, yp01, 0, KB // 2)
            t_half(1, 0)
            mg_k(0, 1, yp01, KB // 2, KB)
            t_half(1, 1)
            mg_tail(0, 1, yp01)

            for ms in range(2, MS):
                mg_full(0, ms)
            for ws in range(1, NS):
                for ms in range(MS):
                    mg_full(ws, ms)

    nc.compile()
    return nc


def _get_graph():
    global _graph_cache
    if _graph_cache is None:
        _graph_cache = _build_graph()
    return _graph_cache


def _prep_inputs(inputs):
    """Host-side: fold scaling into P, cast to bf16, pre-tile to SBUF layout."""
    x = np.asarray(inputs["x"], dtype=np.float32)
    weight = np.asarray(inputs["weight"], dtype=np.float32)
    P = np.asarray(inputs["P"], dtype=np.float32)
    Lambda = np.asarray(inputs["Lambda"], dtype=np.float32)
    Q = np.asarray(inputs["Q"], dtype=np.float32)
    rank_mask = np.asarray(inputs["rank_mask"])

    # Ptilde = P * (s * Lambda * mask); pad rows 64..127 of Ptilde^T with 0
    scale = (SCALING * Lambda * rank_mask.astype(np.float32)).astype(np.float32)
    ptil = (P * scale[None, :]).T  # [RANK, OUT_F]
    pt = np.zeros((P_DIM, OUT_F), dtype=BF16)
    pt[:RANK] = ptil.astype(BF16)

    # Q^T pre-tiled partition-major: [p, kb, r]
    qtp = np.ascontiguousarray(
        Q.T.astype(BF16).reshape(KB, P_DIM, RANK).transpose(1, 0, 2)
    )

    # W pre-tiled per stripe: wt[ns, p, kb, n] = W[ns*512+n, kb*128+p]
    wt = np.ascontiguousarray(
        weight.astype(BF16)
        .T.reshape(KB, P_DIM, NS, N_STRIPE)
        .transpose(2, 1, 0, 3)
    )

    # x per core, per chunk: xt[h, p, kb, m] = x_core[h*512+m, kb*128+p]
    xb = x.astype(BF16)
    in_maps = []
    for c in range(N_CORES):
        xc = xb[c * M_PER : (c + 1) * M_PER]  # [1024, 4096]
        xtc = np.ascontiguousarray(
            xc.reshape(NXC, XH, KB, P_DIM).transpose(0, 3, 2, 1)
        )
        in_maps.append({"xt": xtc, "wt": wt, "qtp": qtp, "pt": pt})
    return in_maps


def run_full(inputs, trace=False, trace_kwargs=None):
    """Run the SPMD kernel on 8 cores. Returns (y_full, BassKernelResults)."""
    in_maps = _prep_inputs(inputs)

    nc = _get_graph()
    last_err = None
    for attempt in range(3):
        try:
            res = run_bass_kernel_spmd(
                nc,
                in_maps,
                core_ids=list(range(N_CORES)),
                trace=trace,
                **(trace_kwargs or {}),
            )
            break
        except Exception as e:  # transient NRT device faults recover on retry
            last_err = e
            time.sleep(10)
    else:
        raise last_err
    y = np.concatenate([res.results[c]["out"] for c in range(N_CORES)], axis=0)
    return y.astype(np.float32, copy=False), res


def _device_available():
    try:
        import jax

        return any("NC" in str(d) or "axon" in str(d).lower() for d in jax.devices())
    except Exception:
        return False


def _run_in_subprocess(inputs):
    # The caller's process may have initialized jax on another platform
    # (e.g. JAX_PLATFORMS=cpu for the reference); run the device pass in a
    # clean child process where jax can pick up the axon/neuron backend.
    import pickle
    import subprocess
    import tempfile

    with tempfile.TemporaryDirectory() as td:
        in_path = os.path.join(td, "in.pkl")
        out_path = os.path.join(td, "out.npy")
        with open(in_path, "wb") as f:
            pickle.dump({k: np.asarray(v) for k, v in inputs.items()}, f)
        env = dict(os.environ)
        env.pop("JAX_PLATFORMS", None)
        env["KERNEL_NO_SUBPROC"] = "1"
        code = (
            "import sys, pickle, numpy as np; "
            f"sys.path.insert(0, {os.path.dirname(os.path.abspath(__file__))!r}); "
            "import kernel; "
            f"inputs = pickle.load(open({in_path!r}, 'rb')); "
            "y, _ = kernel.run_full(inputs, trace=False); "
            f"np.save({out_path!r}, y)"
        )
        subprocess.run([sys.executable, "-c", code], env=env, check=True)
        return np.load(out_path)


def kernel(**inputs) -> np.ndarray:
    if os.environ.get("KERNEL_NO_SUBPROC") != "1":
        if not _device_available():
            return _run_in_subprocess(inputs)
        try:
            y, _ = run_full(inputs, trace=False)
            return y
        except Exception:
            # A wedged device / PJRT client recovers in a fresh process
            # (observed empirically); retry once out-of-process.
            return _run_in_subprocess(inputs)
    y, _ = run_full(inputs, trace=False)
    return y
